# revision 35
# baseline (speedup 1.0000x reference)
"""Masked cross-attention (B=4, NQ=100, HW=4096, D=1024, H=16) on 8 TRN2 cores.

Sharding: kv rows (keys) are split 8 ways; each core runs LayerNorm + K/V
projection on its 512-key slice per batch, computes unnormalized partial
attention for all (b, h) against its keys, all-reduces the softmax
denominators on device, normalizes, and computes a partial out-projection.
The host sums the 8 partial outputs and adds the folded bias.

Schedule: the Tensor-engine instruction stream is software-pipelined so the
PE never waits on ACT/DVE work — batch b's exp/mask runs under batch b+1's
K projection.  The denominator all-reduce is split in two (batches 0-1 and
2-3): the first hides fully under compute, and the tail of the second is
covered by the first half's combine/out-projection/output-DMA.

LayerNorm gamma/beta are folded into the projection weights/biases on the
host; the V-projection bias is folded into the final output bias (exact
because softmax weights sum to one) and the K-projection bias is dropped
entirely (it shifts every key score of a query equally, which softmax
cancels).
"""
import sys

sys.path.insert(0, "/opt/trn_rl_repo")

import numpy as np
import ml_dtypes

import concourse.bacc as bacc
import concourse.bass as bass
import concourse.mybir as mybir
import concourse.tile as tile
from concourse.bass_utils import run_bass_kernel_spmd
from concourse.masks import make_identity
B, NQ, HW, D, H = 4, 100, 4096, 1024, 16
HD = D // H          # 64
NCORE = 8
KC = HW // NCORE     # 512 keys per core per batch
NKT = KC // 128      # 4 key sub-tiles of 128
NDC = D // 128       # 8 chunks of the model dim
EPS = 1e-5
SCALE = 1.0 / np.sqrt(np.float32(HD))  # 1/8

F32 = mybir.dt.float32
BF16 = mybir.dt.bfloat16
AF = mybir.ActivationFunctionType
ALU = mybir.AluOpType

_compiled = {}


def _build():
    nc = bacc.Bacc("TRN2", target_bir_lowering=False, num_devices=NCORE)

    kv_d = nc.dram_tensor("kv", [B, NKT, 128, D], BF16, kind="ExternalInput")
    q_d = nc.dram_tensor("q", [B, NQ, D], BF16, kind="ExternalInput")
    mask_d = nc.dram_tensor("maskT", [128, B, NKT, NQ], BF16, kind="ExternalInput")
    wq_d = nc.dram_tensor("wqT", [128, NDC, D], BF16, kind="ExternalInput")
    wk_d = nc.dram_tensor("wkT", [128, NDC, D], BF16, kind="ExternalInput")
    wv_d = nc.dram_tensor("wvT", [128, NDC, D], BF16, kind="ExternalInput")
    wo_d = nc.dram_tensor("woT", [128, NDC, D], BF16, kind="ExternalInput")
    bq_d = nc.dram_tensor("biasq", [128, NDC], F32, kind="ExternalInput")
    out_d = nc.dram_tensor("out", [128, NDC, B, NQ], BF16, kind="ExternalOutput")

    with tile.TileContext(nc) as tc:
        with (
            tc.tile_pool(name="sb", bufs=1) as sb,
            tc.tile_pool(name="ps", bufs=1, space="PSUM") as ps,
            tc.tile_pool(name="dram", bufs=1, space="DRAM") as dram,
        ):
            # ---- constants ----
            ident = sb.tile([128, 128], BF16, tag="ident")
            make_identity(nc, ident[:])
            eps_t = sb.tile([128, 1], F32, tag="eps")
            nc.vector.memset(eps_t[:], EPS)

            # Weights laid out [128, NDC, D] on the host: one contiguous DMA
            # each.  DMA queue order is managed explicitly below.
            wk_sb = sb.tile([128, NDC, D], BF16, tag="wk")
            wv_sb = sb.tile([128, NDC, D], BF16, tag="wv")
            bqv_sb = sb.tile([128, NDC], F32, tag="bqv")
            bq_sb = [bqv_sb[:, j:j + 1] for j in range(NDC)]
            wq_sb = sb.tile([128, NDC, D], BF16, tag="wq", bufs=1, name="wq")
            wo_sb = sb.tile([128, NDC, D], BF16, tag="wo")

            def layernorm_to_bf16(x_bf16, xn_bf16, p):
                """(x - mean) * rsqrt(var + eps), row-wise over the free dim."""
                stats = sb.tile([128, 2, 6], F32, tag="lnstats", bufs=4)
                nc.vector.bn_stats(stats[:p, 0, :], x_bf16[:p, 0:512])
                nc.vector.bn_stats(stats[:p, 1, :], x_bf16[:p, 512:1024])
                mv = sb.tile([128, 2], F32, tag="lnmv", bufs=4)
                nc.vector.bn_aggr(mv[:p], stats[:p])
                rstd = sb.tile([128, 1], F32, tag="lnrstd", bufs=4)
                nc.scalar.activation(rstd[:p], mv[:p, 1:2], AF.Sqrt, bias=eps_t[:p])
                nc.vector.reciprocal(rstd[:p], rstd[:p])
                nc.vector.tensor_scalar(
                    xn_bf16[:p], x_bf16[:p], mv[:p, 0:1], rstd[:p],
                    ALU.subtract, ALU.mult,
                )

            sloc = [dram.tile([NQ, H], F32, tag=f"sloc{b}", name=f"sloc{b}")
                    for b in range(B)]
            sglob = [dram.tile([NQ, H], F32, tag=f"sglob{b}", name=f"sglob{b}")
                     for b in range(B)]
            # normalized, transposed context for all batches: [p, k, b, q]
            ctxT_all = sb.tile([128, NDC, B, NQ], BF16, tag="ctxTall")
            qnT = sb.tile([128, NDC, B, NQ], BF16, tag="qnT")
            qpT = []

            def load_kv(b):
                """kv loads only (grouped so later transpose waits on the
                sync queue never delay prefetch)."""
                kvraws = []
                for r in range(NKT):
                    kvraw = sb.tile([128, D], BF16, tag="kvraw", bufs=4)
                    nc.sync.dma_start(kvraw[:], kv_d[b, r])
                    kvraws.append(kvraw)
                return kvraws

            def transpose_kv(b, kvraws):
                """LayerNorm + XBAR DMA transpose into kvnT[b].

                kvnT layout: [128 dpart, NKT, NDC, 128 keys], one contiguous
                [NDC, 128] destination slab per key sub-tile so the hardware
                DMA transpose (sync queue) replaces PE transposes + DVE
                copies."""
                kvnT = sb.tile([128, NKT, NDC, 128], BF16, tag="kvnT", bufs=2,
                               name=f"kvnT_{b}")
                for r in range(NKT):
                    layernorm_to_bf16(kvraws[r], kvraws[r], 128)
                    nc.scalar.dma_start_transpose(kvnT[:, r], kvraws[r][:])
                return kvnT

            def kproj(b, kvnT):
                """K projection -> kpT[j]: [128 dout, KC keys] (no bias:
                the K bias cancels in softmax)."""
                kpT = []
                for j in range(NDC):
                    kpT.append(
                        sb.tile([128, KC], BF16, tag=f"kpT{j}", bufs=2,
                                name=f"kpT{j}_{b}")
                    )
                    acc = ps.tile([128, KC], F32, tag="mm", bufs=3)
                    for k in range(NDC):
                        nc.tensor.matmul(
                            acc[:],
                            lhsT=wk_sb[:, k, j * 128:(j + 1) * 128],
                            rhs=kvnT[:, :, k, :],
                            start=(k == 0), stop=(k == NDC - 1),
                        )
                    nc.scalar.activation(kpT[j][:], acc[:], AF.Copy)
                return kpT

            def vproj(b, kvnT):
                """V projection -> vp_ext[r]: [128 keys, H, HD+1], col HD=1."""
                vp_ext = []
                for r in range(NKT):
                    vpe = sb.tile([128, H, HD + 1], BF16, tag=f"vpe{r}", bufs=2,
                                  name=f"vpe{r}_{b}")
                    vp_ext.append(vpe)
                    nc.vector.memset(vpe[:, :, HD:HD + 1], 1.0)
                    for nh in range(2):
                        acc = ps.tile([128, 512], F32, tag="mm", bufs=3)
                        for k in range(NDC):
                            nc.tensor.matmul(
                                acc[:],
                                lhsT=kvnT[:, r, k, :],
                                rhs=wv_sb[:, k, nh * 512:(nh + 1) * 512],
                                start=(k == 0), stop=(k == NDC - 1),
                            )
                        if nh == 0:
                            nc.vector.tensor_copy(
                                out=vpe[:, nh * 8:(nh + 1) * 8, 0:HD],
                                in_=acc[:].rearrange("p (g d) -> p g d", g=8),
                            )
                        else:
                            nc.scalar.activation(
                                vpe[:, nh * 8:(nh + 1) * 8, 0:HD],
                                acc[:].rearrange("p (g d) -> p g d", g=8),
                                AF.Copy,
                            )
                return vp_ext

            def load_q():
                qraws = []
                for b in range(B):
                    qraw = sb.tile([NQ, D], BF16, tag="qraw", bufs=3)
                    nc.sync.dma_start(qraw[:], q_d[b])
                    qraws.append(qraw)
                return qraws

            def ln_T_q(qraws):
                """LayerNorm + PE transpose of q, all batches (the early PE
                transposes also give the Tensor engine work while the first
                K projection's inputs stream in)."""
                for b in range(B):
                    qn = sb.tile([NQ, D], BF16, tag="qn", bufs=1)
                    layernorm_to_bf16(qraws[b], qn, NQ)
                    for k4 in range(NDC // 4):
                        tr = ps.tile([128, 4, NQ], BF16, tag="small", bufs=3)
                        for kk in range(4):
                            k = 4 * k4 + kk
                            nc.tensor.transpose(
                                tr[:, kk, :], qn[:, k * 128:(k + 1) * 128],
                                ident[:NQ, :NQ],
                            )
                        nc.vector.tensor_copy(
                            out=qnT[:, 4 * k4:4 * k4 + 4, b, :], in_=tr[:]
                        )

            def qproj():
                """qpT_pad[j]: [128, B, 2, NQ] block-diagonal by head: rows
                0:64 hold head 2j over i=0 columns, rows 64:128 hold head
                2j+1 over i=1 columns, zeros elsewhere."""
                for j in range(NDC):
                    qpT.append(
                        sb.tile([128, B, 2, NQ], BF16, tag=f"qpT{j}",
                                name=f"qpT{j}")
                    )
                    nc.gpsimd.memset(qpT[j][:], 0.0)
                    acc = ps.tile([128, B * NQ], F32, tag="sc", bufs=2)
                    for k in range(NDC):
                        nc.tensor.matmul(
                            acc[:],
                            lhsT=wq_sb[:, k, j * 128:(j + 1) * 128],
                            rhs=qnT[:, k, :, :].rearrange("p b q -> p (b q)"),
                            start=(k == 0), stop=(k == NDC - 1),
                        )
                    nc.scalar.activation(
                        qpT[j][0:HD, :, 0, :],
                        acc[0:HD, :].rearrange("p (b q) -> p b q", b=B),
                        AF.Identity, bias=bq_sb[j][0:HD],
                    )
                    nc.scalar.activation(
                        qpT[j][HD:128, :, 1, :],
                        acc[HD:128, :].rearrange("p (b q) -> p b q", b=B),
                        AF.Identity, bias=bq_sb[j][HD:128],
                    )

            def scores_exp(b, kpT, mask_b):
                """scores^T + exp + mask for all heads of batch b."""
                exp_all = sb.tile([128, NKT, H, NQ], BF16, tag="expall", bufs=2,
                                  name=f"exp_{b}")
                for j in range(NDC):
                    for c2 in range(2):
                        sc = ps.tile([128, 2, 2, NQ], F32, tag="sc", bufs=2)
                        for cc in range(2):
                            c = 2 * c2 + cc
                            nc.tensor.matmul(
                                sc[:, cc, :, :].rearrange("p i q -> p (i q)"),
                                lhsT=kpT[j][:, c * 128:(c + 1) * 128],
                                rhs=qpT[j][:, b, :, :].rearrange(
                                    "p i q -> p (i q)"),
                                start=True, stop=True,
                            )
                        nc.scalar.activation(
                            exp_all[:, 2 * c2:2 * c2 + 2, 2 * j:2 * j + 2, :],
                            sc[:], AF.Exp, scale=float(SCALE),
                        )
                    for hh in range(2):
                        h = 2 * j + hh
                        eng = nc.vector if hh == 0 else nc.gpsimd
                        eng.tensor_mul(
                            exp_all[:, :, h, :], exp_all[:, :, h, :], mask_b
                        )
                return exp_all

            def ctx_block(b, exp_all, vp_ext):
                """Unnormalized ctx + denominators for batch b; DMA the local
                denominators out for the all-reduce."""
                ctx_b = sb.tile([NQ, H, HD + 1], F32, tag="ctxsb", bufs=4,
                                name=f"ctx_{b}")
                for h in range(H):
                    ctx_ps = ps.tile([NQ, HD + 1], F32, tag="small", bufs=3)
                    for c in range(NKT):
                        nc.tensor.matmul(
                            ctx_ps[:],
                            lhsT=exp_all[:, c, h, :],
                            rhs=vp_ext[c][:, h, :],
                            start=(c == 0), stop=(c == NKT - 1),
                        )
                    if h % 2 == 0:
                        nc.vector.tensor_copy(out=ctx_b[:, h, :], in_=ctx_ps[:])
                    else:
                        nc.scalar.activation(ctx_b[:, h, :], ctx_ps[:], AF.Copy)
                # pack the strided denominator column so its DMA moves 64B
                # runs instead of 4B elements (the DMA engine is descriptor-
                # latency-bound on tiny runs and would delay the all-reduce)
                den = sb.tile([NQ, H], F32, tag="den", bufs=2)
                nc.gpsimd.tensor_copy(out=den[:], in_=ctx_b[:, :, HD])
                nc.sync.dma_start(sloc[b][:], den[:])
                return ctx_b

            def allreduce_b(b):
                """Per-batch denominator all-reduce.  One collective per
                batch: the first fires ~100us before its result is needed and
                absorbs cross-core launch skew; the rest are cheap syncs.
                The recip DMA rides the sync queue so its completion wait
                never blocks the gpsimd queue."""
                nc.gpsimd.collective_compute(
                    "AllReduce", ALU.add,
                    replica_groups=[list(range(NCORE))],
                    ins=[sloc[b][:].opt()], outs=[sglob[b][:].opt()],
                )
                return b

            def recip_fetch(b):
                """Fetch the all-reduced denominators for batch b.  Must be
                emitted BEFORE any later collective_compute: DMA-after-
                collective ordering uses one program-order CC counter."""
                recip = sb.tile([NQ, H], F32, tag="recip", bufs=4,
                                name=f"recip_{b}")
                nc.scalar.dma_start(recip[:], sglob[b][:])
                return recip

            def combine_block(b, ctx_b, recip):
                """Normalize by global denominators and XBAR-transpose into
                ctxT_all.  Scales alternate DVE/ACT by head."""
                nc.vector.reciprocal(recip[:], recip[:])
                ctxn = sb.tile([NQ, H, HD], BF16, tag="ctxn", bufs=2)
                for h in range(H):
                    if h % 2 == 0:
                        nc.vector.tensor_scalar_mul(
                            ctxn[:, h, :], ctx_b[:, h, 0:HD],
                            recip[:, h:h + 1]
                        )
                    else:
                        nc.scalar.activation(
                            ctxn[:, h, :], ctx_b[:, h, 0:HD], AF.Copy,
                            scale=recip[:, h:h + 1],
                        )
                for j4 in range(2):
                    tr = ps.tile([128, 4, NQ], BF16, tag="small", bufs=3)
                    for jj in range(4):
                        for hh in range(2):
                            nc.tensor.transpose(
                                tr[hh * HD:(hh + 1) * HD, jj, :],
                                ctxn[:, 2 * (4 * j4 + jj) + hh, :],
                                ident[:NQ, :NQ],
                            )
                    nc.vector.tensor_copy(
                        out=ctxT_all[:, 4 * j4:4 * j4 + 4, b, :], in_=tr[:]
                    )

            # reuses wq's slot (wq is dead after the Q projection)
            out_sb = sb.tile([128, NDC, B, NQ], BF16, tag="wq", bufs=1,
                             name="out_sb")

            def outproj(b0, nb):
                """Out-projection for batches [b0, b0+nb), then DMA that
                slice of the output."""
                for m in range(NDC):
                    acc = ps.tile([128, nb, NQ], F32, tag="sc", bufs=2)
                    for k in range(NDC):
                        nc.tensor.matmul(
                            acc[:],
                            lhsT=wo_sb[:, k, m * 128:(m + 1) * 128],
                            rhs=ctxT_all[:, k, b0:b0 + nb, :].rearrange(
                                "p b q -> p (b q)"),
                            start=(k == 0), stop=(k == NDC - 1),
                        )
                    if m % 2 == 0:
                        nc.vector.tensor_copy(
                            out=out_sb[:, m, b0:b0 + nb, :], in_=acc[:]
                        )
                    else:
                        nc.scalar.activation(
                            out_sb[:, m, b0:b0 + nb, :], acc[:], AF.Copy,
                        )
                nc.sync.dma_start(
                    out_d[:, :, b0:b0 + nb, :], out_sb[:, :, b0:b0 + nb, :]
                )

            mask_all = sb.tile([128, B, NKT, NQ], BF16, tag="maskb")

            def load_mask():
                nc.sync.dma_start(mask_all[:], mask_d[:])

            # ---- pipelined schedule ----
            # Sync queue: kv(b0), wk, q, wq, bq, wv, mask0 all dispatched
            # before the first (LN-gated) XBAR transpose so nothing blocks
            # prefetch.  PE queue: Kproj(b0), qT, Qproj, Vproj(b0), then per
            # batch: scores(b) | Kproj(b+1) | ctx(b) | Vproj(b+1).
            kvraws0 = load_kv(0)
            nc.sync.dma_start(wk_sb[:], wk_d[:])
            nc.sync.dma_start(wq_sb[:], wq_d[:])
            nc.sync.dma_start(bqv_sb[:], bq_d[:])
            nc.sync.dma_start(wv_sb[:], wv_d[:])
            load_mask()
            kvnT0 = transpose_kv(0, kvraws0)
            qraws = load_q()

            kpT0 = kproj(0, kvnT0)
            ln_T_q(qraws)
            qproj()
            kvraws1 = load_kv(1)
            nc.sync.dma_start(wo_sb[:], wo_d[:])
            kvnT1 = transpose_kv(1, kvraws1)
            vp0 = vproj(0, kvnT0)

            exp0 = scores_exp(0, kpT0, mask_all[:, 0])
            kpT1 = kproj(1, kvnT1)
            ctx0 = ctx_block(0, exp0, vp0)
            recip0 = allreduce_b(0)
            kvraws2 = load_kv(2)
            kvnT2 = transpose_kv(2, kvraws2)
            vp1 = vproj(1, kvnT1)

            exp1 = scores_exp(1, kpT1, mask_all[:, 1])
            kpT2 = kproj(2, kvnT2)
            ctx1 = ctx_block(1, exp1, vp1)
            recip1 = allreduce_b(1)
            kvraws3 = load_kv(3)
            kvnT3 = transpose_kv(3, kvraws3)
            vp2 = vproj(2, kvnT2)

            exp2 = scores_exp(2, kpT2, mask_all[:, 2])
            kpT3 = kproj(3, kvnT3)
            ctx2 = ctx_block(2, exp2, vp2)
            recip2 = allreduce_b(2)
            vp3 = vproj(3, kvnT3)

            exp3 = scores_exp(3, kpT3, mask_all[:, 3])
            r0 = recip_fetch(0)
            r1 = recip_fetch(1)
            r2 = recip_fetch(2)
            ctx3 = ctx_block(3, exp3, vp3)
            allreduce_b(3)
            combine_block(0, ctx0, r0)
            combine_block(1, ctx1, r1)
            outproj(0, 2)
            combine_block(2, ctx2, r2)
            outproj(2, 1)
            r3 = recip_fetch(3)
            combine_block(3, ctx3, r3)
            outproj(3, 1)

    nc.compile()
    return nc


def _prep_in_maps(q, kv, mask, in_proj_w, in_proj_b, out_w, out_b,
                  g_q, b_q, g_kv, b_kv):
    """Host-side prep: fold LN affine + V-bias, shard kv/mask per core.

    Returns (in_maps, bias_total)."""
    q = np.asarray(q, np.float32)
    kv = np.asarray(kv, np.float32)
    mask = np.asarray(mask)
    in_proj_w = np.asarray(in_proj_w, np.float32)
    in_proj_b = np.asarray(in_proj_b, np.float32)
    out_w = np.asarray(out_w, np.float32)
    out_b = np.asarray(out_b, np.float32)
    g_q = np.asarray(g_q, np.float32)
    b_q = np.asarray(b_q, np.float32)
    g_kv = np.asarray(g_kv, np.float32)
    b_kv = np.asarray(b_kv, np.float32)

    Wq, Wk, Wv = in_proj_w[:D], in_proj_w[D:2 * D], in_proj_w[2 * D:]
    bq, bk, bv = in_proj_b[:D], in_proj_b[D:2 * D], in_proj_b[2 * D:]

    # Fold LayerNorm affine into projections: LN(x)*g+b @ W^T + c
    #   = LN(x) @ (W*g)^T + (W@b + c)
    WqT = (Wq * g_q[None, :]).T.astype(ml_dtypes.bfloat16)
    WkT = (Wk * g_kv[None, :]).T.astype(ml_dtypes.bfloat16)
    WvT = (Wv * g_kv[None, :]).T.astype(ml_dtypes.bfloat16)
    bq_eff = (bq + Wq @ b_q).astype(np.float32)
    bv_eff = (bv + Wv @ b_kv).astype(np.float32)
    # The K bias (bk + Wk@b_kv) shifts all key scores of a query equally and
    # cancels in softmax; it is dropped.  The V bias passes through softmax
    # unchanged (weights sum to 1): fold into the final output bias.
    WoT = out_w.T.astype(ml_dtypes.bfloat16)
    bias_total = (out_b + out_w @ bv_eff).astype(np.float32)

    # per-query key mask; all-zero mask rows attend everywhere
    kv16 = kv.astype(ml_dtypes.bfloat16)
    allowed = (mask != 0)
    has_any = allowed.any(axis=-1, keepdims=True)
    eff = np.where(has_any, allowed, True)  # [B, NQ, HW] bool

    common = {
        "q": np.ascontiguousarray(q.astype(ml_dtypes.bfloat16)),
        "wqT": np.ascontiguousarray(WqT.reshape(NDC, 128, D).transpose(1, 0, 2)),
        "wkT": np.ascontiguousarray(WkT.reshape(NDC, 128, D).transpose(1, 0, 2)),
        "wvT": np.ascontiguousarray(WvT.reshape(NDC, 128, D).transpose(1, 0, 2)),
        "woT": np.ascontiguousarray(WoT.reshape(NDC, 128, D).transpose(1, 0, 2)),
        "biasq": np.ascontiguousarray(bq_eff.reshape(NDC, 128).T),
    }
    in_maps = []
    for c in range(NCORE):
        sl = slice(c * KC, (c + 1) * KC)
        kv_c = kv16[:, sl, :].reshape(B, NKT, 128, D)
        # mask slice -> [128, B, NKT, NQ] bf16 (keysub-tile on partitions)
        m_c = eff[:, :, sl].transpose(0, 2, 1).reshape(B, NKT, 128, NQ)
        m_c = m_c.transpose(2, 0, 1, 3).astype(ml_dtypes.bfloat16)
        in_maps.append({
            **common,
            "kv": np.ascontiguousarray(kv_c),
            "maskT": np.ascontiguousarray(m_c),
        })
    return in_maps, bias_total


def kernel(q, kv, mask, in_proj_w, in_proj_b, out_w, out_b, g_q, b_q, g_kv, b_kv):
    in_maps, bias_total = _prep_in_maps(
        q, kv, mask, in_proj_w, in_proj_b, out_w, out_b, g_q, b_q, g_kv, b_kv
    )
    if "nc" not in _compiled:
        _compiled["nc"] = _build()
    nc = _compiled["nc"]

    res = run_bass_kernel_spmd(nc, in_maps, core_ids=list(range(NCORE)))

    out = np.zeros((B, NQ, D), np.float32)
    for c in range(NCORE):
        part = res.results[c]["out"]  # [128 p, NDC m, B, NQ]; dout = m*128+p
        out += part.transpose(2, 3, 1, 0).reshape(B, NQ, D).astype(np.float32)
    out += bias_total[None, None, :]
    return out


# revision 36
# speedup vs baseline: 1.0033x; 1.0033x over previous
"""Masked cross-attention (B=4, NQ=100, HW=4096, D=1024, H=16) on 8 TRN2 cores.

Sharding: kv rows (keys) are split 8 ways; each core runs LayerNorm + K/V
projection on its 512-key slice per batch, computes unnormalized partial
attention for all (b, h) against its keys, all-reduces the softmax
denominators on device, normalizes, and computes a partial out-projection.
The host sums the 8 partial outputs and adds the folded bias.

Schedule: the Tensor-engine instruction stream is software-pipelined so the
PE never waits on ACT/DVE work — batch b's exp/mask runs under batch b+1's
K projection.  The denominator all-reduce is split in two (batches 0-1 and
2-3): the first hides fully under compute, and the tail of the second is
covered by the first half's combine/out-projection/output-DMA.

LayerNorm gamma/beta are folded into the projection weights/biases on the
host; the V-projection bias is folded into the final output bias (exact
because softmax weights sum to one) and the K-projection bias is dropped
entirely (it shifts every key score of a query equally, which softmax
cancels).
"""
import sys

sys.path.insert(0, "/opt/trn_rl_repo")

import numpy as np
import ml_dtypes

import concourse.bacc as bacc
import concourse.bass as bass
import concourse.mybir as mybir
import concourse.tile as tile
from concourse.bass_utils import run_bass_kernel_spmd
B, NQ, HW, D, H = 4, 100, 4096, 1024, 16
HD = D // H          # 64
NCORE = 8
KC = HW // NCORE     # 512 keys per core per batch
NKT = KC // 128      # 4 key sub-tiles of 128
NDC = D // 128       # 8 chunks of the model dim
EPS = 1e-5
SCALE = 1.0 / np.sqrt(np.float32(HD))  # 1/8

F32 = mybir.dt.float32
BF16 = mybir.dt.bfloat16
AF = mybir.ActivationFunctionType
ALU = mybir.AluOpType

_compiled = {}


def _build():
    nc = bacc.Bacc("TRN2", target_bir_lowering=False, num_devices=NCORE)

    kv_d = nc.dram_tensor("kv", [B, NKT, 128, D], BF16, kind="ExternalInput")
    q_d = nc.dram_tensor("q", [B, NQ, D], BF16, kind="ExternalInput")
    mask_d = nc.dram_tensor("maskT", [128, B, NKT, NQ], BF16, kind="ExternalInput")
    wq_d = nc.dram_tensor("wqT", [128, NDC, D], BF16, kind="ExternalInput")
    wk_d = nc.dram_tensor("wkT", [128, NDC, D], BF16, kind="ExternalInput")
    wv_d = nc.dram_tensor("wvT", [128, NDC, D], BF16, kind="ExternalInput")
    wo_d = nc.dram_tensor("woT", [128, NDC, D], BF16, kind="ExternalInput")
    bq_d = nc.dram_tensor("biasq", [128, NDC], F32, kind="ExternalInput")
    out_d = nc.dram_tensor("out", [128, NDC, B, NQ], BF16, kind="ExternalOutput")

    with tile.TileContext(nc) as tc:
        with (
            tc.tile_pool(name="sb", bufs=1) as sb,
            tc.tile_pool(name="ps", bufs=1, space="PSUM") as ps,
            tc.tile_pool(name="dram", bufs=1, space="DRAM") as dram,
        ):
            # ---- constants ----
            eps_t = sb.tile([128, 1], F32, tag="eps")
            nc.vector.memset(eps_t[:], EPS)

            # Weights laid out [128, NDC, D] on the host: one contiguous DMA
            # each.  DMA queue order is managed explicitly below.
            wk_sb = sb.tile([128, NDC, D], BF16, tag="wk")
            wv_sb = sb.tile([128, NDC, D], BF16, tag="wv")
            bqv_sb = sb.tile([128, NDC], F32, tag="bqv")
            bq_sb = [bqv_sb[:, j:j + 1] for j in range(NDC)]
            wq_sb = sb.tile([128, NDC, D], BF16, tag="wq", bufs=1, name="wq")
            wo_sb = sb.tile([128, NDC, D], BF16, tag="wo")

            def layernorm_to_bf16(x_bf16, xn_bf16, p):
                """(x - mean) * rsqrt(var + eps), row-wise over the free dim."""
                stats = sb.tile([128, 2, 6], F32, tag="lnstats", bufs=4)
                nc.vector.bn_stats(stats[:p, 0, :], x_bf16[:p, 0:512])
                nc.vector.bn_stats(stats[:p, 1, :], x_bf16[:p, 512:1024])
                mv = sb.tile([128, 2], F32, tag="lnmv", bufs=4)
                nc.vector.bn_aggr(mv[:p], stats[:p])
                rstd = sb.tile([128, 1], F32, tag="lnrstd", bufs=4)
                nc.scalar.activation(rstd[:p], mv[:p, 1:2], AF.Sqrt, bias=eps_t[:p])
                nc.vector.reciprocal(rstd[:p], rstd[:p])
                nc.vector.tensor_scalar(
                    xn_bf16[:p], x_bf16[:p], mv[:p, 0:1], rstd[:p],
                    ALU.subtract, ALU.mult,
                )

            sloc = [dram.tile([NQ, H], F32, tag=f"sloc{b}", name=f"sloc{b}")
                    for b in range(B)]
            sglob = [dram.tile([NQ, H], F32, tag=f"sglob{b}", name=f"sglob{b}")
                     for b in range(B)]
            # normalized, transposed context for all batches: [p, k, b, q]
            ctxT_all = sb.tile([128, B, NDC, 112], BF16, tag="ctxTall")
            NQP = 112  # q rows padded to the XBAR 16-row granule
            qnT = sb.tile([128, B, NDC, NQP], BF16, tag="qnT")
            qpT = []

            def load_kv(b):
                """kv loads only (grouped so later transpose waits on the
                sync queue never delay prefetch)."""
                kvraws = []
                for r in range(NKT):
                    kvraw = sb.tile([128, D], BF16, tag="kvraw", bufs=4)
                    nc.sync.dma_start(kvraw[:], kv_d[b, r])
                    kvraws.append(kvraw)
                return kvraws

            def transpose_kv(b, kvraws):
                """LayerNorm + XBAR DMA transpose into kvnT[b].

                kvnT layout: [128 dpart, NKT, NDC, 128 keys], one contiguous
                [NDC, 128] destination slab per key sub-tile so the hardware
                DMA transpose (sync queue) replaces PE transposes + DVE
                copies."""
                kvnT = sb.tile([128, NKT, NDC, 128], BF16, tag="kvnT", bufs=2,
                               name=f"kvnT_{b}")
                for r in range(NKT):
                    layernorm_to_bf16(kvraws[r], kvraws[r], 128)
                    nc.scalar.dma_start_transpose(kvnT[:, r], kvraws[r][:])
                return kvnT

            def kproj(b, kvnT):
                """K projection -> kpT[j]: [128 dout, KC keys] (no bias:
                the K bias cancels in softmax)."""
                kpT = []
                for j in range(NDC):
                    kpT.append(
                        sb.tile([128, KC], BF16, tag=f"kpT{j}", bufs=2,
                                name=f"kpT{j}_{b}")
                    )
                    acc = ps.tile([128, KC], F32, tag="mm", bufs=3)
                    for k in range(NDC):
                        nc.tensor.matmul(
                            acc[:],
                            lhsT=wk_sb[:, k, j * 128:(j + 1) * 128],
                            rhs=kvnT[:, :, k, :],
                            start=(k == 0), stop=(k == NDC - 1),
                        )
                    nc.scalar.activation(kpT[j][:], acc[:], AF.Copy)
                return kpT

            def vproj(b, kvnT):
                """V projection -> vp_ext[r]: [128 keys, H, HD+1], col HD=1."""
                vp_ext = []
                for r in range(NKT):
                    vpe = sb.tile([128, H, HD + 1], BF16, tag=f"vpe{r}", bufs=2,
                                  name=f"vpe{r}_{b}")
                    vp_ext.append(vpe)
                    nc.vector.memset(vpe[:, :, HD:HD + 1], 1.0)
                    for nh in range(2):
                        acc = ps.tile([128, 512], F32, tag="mm", bufs=3)
                        for k in range(NDC):
                            nc.tensor.matmul(
                                acc[:],
                                lhsT=kvnT[:, r, k, :],
                                rhs=wv_sb[:, k, nh * 512:(nh + 1) * 512],
                                start=(k == 0), stop=(k == NDC - 1),
                            )
                        if nh == 0:
                            nc.vector.tensor_copy(
                                out=vpe[:, nh * 8:(nh + 1) * 8, 0:HD],
                                in_=acc[:].rearrange("p (g d) -> p g d", g=8),
                            )
                        else:
                            nc.scalar.activation(
                                vpe[:, nh * 8:(nh + 1) * 8, 0:HD],
                                acc[:].rearrange("p (g d) -> p g d", g=8),
                                AF.Copy,
                            )
                return vp_ext

            def load_q():
                qraws = []
                for b in range(B):
                    qraw = sb.tile([NQ, D], BF16, tag="qraw", bufs=3)
                    nc.sync.dma_start(qraw[:], q_d[b])
                    qraws.append(qraw)
                return qraws

            def ln_T_q(qraws):
                """LayerNorm + XBAR transpose of q, all batches (rows
                100:112 of the padded tile are never read downstream)."""
                for b in range(B):
                    qn = sb.tile([NQP, D], BF16, tag="qn", bufs=1)
                    layernorm_to_bf16(qraws[b], qn, NQ)
                    nc.scalar.dma_start_transpose(qnT[:, b], qn[:])

            def qproj():
                """qpT_pad[j]: [128, B, 2, NQ] block-diagonal by head: rows
                0:64 hold head 2j over i=0 columns, rows 64:128 hold head
                2j+1 over i=1 columns, zeros elsewhere."""
                for j in range(NDC):
                    qpT.append(
                        sb.tile([128, B, 2, NQ], BF16, tag=f"qpT{j}",
                                name=f"qpT{j}")
                    )
                    nc.gpsimd.memset(qpT[j][:], 0.0)
                    acc = ps.tile([128, B * NQ], F32, tag="sc", bufs=2)
                    for k in range(NDC):
                        nc.tensor.matmul(
                            acc[:],
                            lhsT=wq_sb[:, k, j * 128:(j + 1) * 128],
                            rhs=qnT[:, :, k, 0:NQ],
                            start=(k == 0), stop=(k == NDC - 1),
                        )
                    nc.scalar.activation(
                        qpT[j][0:HD, :, 0, :],
                        acc[0:HD, :].rearrange("p (b q) -> p b q", b=B),
                        AF.Identity, bias=bq_sb[j][0:HD],
                    )
                    nc.scalar.activation(
                        qpT[j][HD:128, :, 1, :],
                        acc[HD:128, :].rearrange("p (b q) -> p b q", b=B),
                        AF.Identity, bias=bq_sb[j][HD:128],
                    )

            def scores_exp(b, kpT, mask_b):
                """scores^T + exp + mask for all heads of batch b."""
                exp_all = sb.tile([128, NKT, H, NQ], BF16, tag="expall", bufs=2,
                                  name=f"exp_{b}")
                for j in range(NDC):
                    for c2 in range(2):
                        sc = ps.tile([128, 2, 2, NQ], F32, tag="sc", bufs=2)
                        for cc in range(2):
                            c = 2 * c2 + cc
                            nc.tensor.matmul(
                                sc[:, cc, :, :].rearrange("p i q -> p (i q)"),
                                lhsT=kpT[j][:, c * 128:(c + 1) * 128],
                                rhs=qpT[j][:, b, :, :].rearrange(
                                    "p i q -> p (i q)"),
                                start=True, stop=True,
                            )
                        nc.scalar.activation(
                            exp_all[:, 2 * c2:2 * c2 + 2, 2 * j:2 * j + 2, :],
                            sc[:], AF.Exp, scale=float(SCALE),
                        )
                    for hh in range(2):
                        h = 2 * j + hh
                        eng = nc.vector if hh == 0 else nc.gpsimd
                        eng.tensor_mul(
                            exp_all[:, :, h, :], exp_all[:, :, h, :], mask_b
                        )
                return exp_all

            def ctx_block(b, exp_all, vp_ext):
                """Unnormalized ctx + denominators for batch b; DMA the local
                denominators out for the all-reduce."""
                ctx_b = sb.tile([NQ, H, HD + 1], F32, tag="ctxsb", bufs=4,
                                name=f"ctx_{b}")
                for h in range(H):
                    ctx_ps = ps.tile([NQ, HD + 1], F32, tag="small", bufs=3)
                    for c in range(NKT):
                        nc.tensor.matmul(
                            ctx_ps[:],
                            lhsT=exp_all[:, c, h, :],
                            rhs=vp_ext[c][:, h, :],
                            start=(c == 0), stop=(c == NKT - 1),
                        )
                    if h % 2 == 0:
                        nc.vector.tensor_copy(out=ctx_b[:, h, :], in_=ctx_ps[:])
                    else:
                        nc.scalar.activation(ctx_b[:, h, :], ctx_ps[:], AF.Copy)
                # pack the strided denominator column so its DMA moves 64B
                # runs instead of 4B elements (the DMA engine is descriptor-
                # latency-bound on tiny runs and would delay the all-reduce)
                den = sb.tile([NQ, H], F32, tag="den", bufs=2)
                nc.gpsimd.tensor_copy(out=den[:], in_=ctx_b[:, :, HD])
                nc.sync.dma_start(sloc[b][:], den[:])
                return ctx_b

            def allreduce_b(b):
                """Per-batch denominator all-reduce.  One collective per
                batch: the first fires ~100us before its result is needed and
                absorbs cross-core launch skew; the rest are cheap syncs.
                The recip DMA rides the sync queue so its completion wait
                never blocks the gpsimd queue."""
                nc.gpsimd.collective_compute(
                    "AllReduce", ALU.add,
                    replica_groups=[list(range(NCORE))],
                    ins=[sloc[b][:].opt()], outs=[sglob[b][:].opt()],
                )
                return b

            def recip_fetch(b):
                """Fetch the all-reduced denominators for batch b.  Must be
                emitted BEFORE any later collective_compute: DMA-after-
                collective ordering uses one program-order CC counter."""
                recip = sb.tile([NQ, H], F32, tag="recip", bufs=4,
                                name=f"recip_{b}")
                nc.scalar.dma_start(recip[:], sglob[b][:])
                return recip

            def combine_block(b, ctx_b, recip):
                """Normalize by global denominators and XBAR-transpose into
                ctxT_all.  Scales alternate DVE/ACT by head."""
                nc.vector.reciprocal(recip[:], recip[:])
                ctxn = sb.tile([112, H, HD], BF16, tag="ctxn", bufs=1)
                for h in range(H):
                    if h % 2 == 0:
                        nc.vector.tensor_scalar_mul(
                            ctxn[:NQ, h, :], ctx_b[:, h, 0:HD],
                            recip[:, h:h + 1]
                        )
                    else:
                        nc.scalar.activation(
                            ctxn[:NQ, h, :], ctx_b[:, h, 0:HD], AF.Copy,
                            scale=recip[:, h:h + 1],
                        )
                nc.scalar.dma_start_transpose(ctxT_all[:, b], ctxn[:])

            # reuses wq's slot (wq is dead after the Q projection)
            out_sb = sb.tile([128, NDC, B, NQ], BF16, tag="wq", bufs=1,
                             name="out_sb")

            def outproj(b0, nb):
                """Out-projection for batches [b0, b0+nb), then DMA that
                slice of the output."""
                for m in range(NDC):
                    acc = ps.tile([128, nb, NQ], F32, tag="sc", bufs=2)
                    for k in range(NDC):
                        nc.tensor.matmul(
                            acc[:],
                            lhsT=wo_sb[:, k, m * 128:(m + 1) * 128],
                            rhs=ctxT_all[:, b0:b0 + nb, k, 0:NQ],
                            start=(k == 0), stop=(k == NDC - 1),
                        )
                    if m % 2 == 0:
                        nc.vector.tensor_copy(
                            out=out_sb[:, m, b0:b0 + nb, :], in_=acc[:]
                        )
                    else:
                        nc.scalar.activation(
                            out_sb[:, m, b0:b0 + nb, :], acc[:], AF.Copy,
                        )
                nc.sync.dma_start(
                    out_d[:, :, b0:b0 + nb, :], out_sb[:, :, b0:b0 + nb, :]
                )

            mask_all = sb.tile([128, B, NKT, NQ], BF16, tag="maskb")

            def load_mask():
                nc.sync.dma_start(mask_all[:], mask_d[:])

            # ---- pipelined schedule ----
            # Sync queue: kv(b0), wk, q, wq, bq, wv, mask0 all dispatched
            # before the first (LN-gated) XBAR transpose so nothing blocks
            # prefetch.  PE queue: Kproj(b0), qT, Qproj, Vproj(b0), then per
            # batch: scores(b) | Kproj(b+1) | ctx(b) | Vproj(b+1).
            kvraws0 = load_kv(0)
            nc.sync.dma_start(wk_sb[:], wk_d[:])
            nc.sync.dma_start(wq_sb[:], wq_d[:])
            nc.sync.dma_start(bqv_sb[:], bq_d[:])
            nc.sync.dma_start(wv_sb[:], wv_d[:])
            load_mask()
            kvnT0 = transpose_kv(0, kvraws0)
            qraws = load_q()

            kpT0 = kproj(0, kvnT0)
            ln_T_q(qraws)
            qproj()
            kvraws1 = load_kv(1)
            nc.sync.dma_start(wo_sb[:], wo_d[:])
            kvnT1 = transpose_kv(1, kvraws1)
            vp0 = vproj(0, kvnT0)

            exp0 = scores_exp(0, kpT0, mask_all[:, 0])
            kpT1 = kproj(1, kvnT1)
            ctx0 = ctx_block(0, exp0, vp0)
            recip0 = allreduce_b(0)
            kvraws2 = load_kv(2)
            kvnT2 = transpose_kv(2, kvraws2)
            vp1 = vproj(1, kvnT1)

            exp1 = scores_exp(1, kpT1, mask_all[:, 1])
            kpT2 = kproj(2, kvnT2)
            ctx1 = ctx_block(1, exp1, vp1)
            recip1 = allreduce_b(1)
            kvraws3 = load_kv(3)
            kvnT3 = transpose_kv(3, kvraws3)
            vp2 = vproj(2, kvnT2)

            exp2 = scores_exp(2, kpT2, mask_all[:, 2])
            kpT3 = kproj(3, kvnT3)
            ctx2 = ctx_block(2, exp2, vp2)
            recip2 = allreduce_b(2)
            vp3 = vproj(3, kvnT3)

            exp3 = scores_exp(3, kpT3, mask_all[:, 3])
            r0 = recip_fetch(0)
            r1 = recip_fetch(1)
            r2 = recip_fetch(2)
            ctx3 = ctx_block(3, exp3, vp3)
            allreduce_b(3)
            combine_block(0, ctx0, r0)
            combine_block(1, ctx1, r1)
            outproj(0, 2)
            combine_block(2, ctx2, r2)
            outproj(2, 1)
            r3 = recip_fetch(3)
            combine_block(3, ctx3, r3)
            outproj(3, 1)

    nc.compile()
    return nc


def _prep_in_maps(q, kv, mask, in_proj_w, in_proj_b, out_w, out_b,
                  g_q, b_q, g_kv, b_kv):
    """Host-side prep: fold LN affine + V-bias, shard kv/mask per core.

    Returns (in_maps, bias_total)."""
    q = np.asarray(q, np.float32)
    kv = np.asarray(kv, np.float32)
    mask = np.asarray(mask)
    in_proj_w = np.asarray(in_proj_w, np.float32)
    in_proj_b = np.asarray(in_proj_b, np.float32)
    out_w = np.asarray(out_w, np.float32)
    out_b = np.asarray(out_b, np.float32)
    g_q = np.asarray(g_q, np.float32)
    b_q = np.asarray(b_q, np.float32)
    g_kv = np.asarray(g_kv, np.float32)
    b_kv = np.asarray(b_kv, np.float32)

    Wq, Wk, Wv = in_proj_w[:D], in_proj_w[D:2 * D], in_proj_w[2 * D:]
    bq, bk, bv = in_proj_b[:D], in_proj_b[D:2 * D], in_proj_b[2 * D:]

    # Fold LayerNorm affine into projections: LN(x)*g+b @ W^T + c
    #   = LN(x) @ (W*g)^T + (W@b + c)
    WqT = (Wq * g_q[None, :]).T.astype(ml_dtypes.bfloat16)
    WkT = (Wk * g_kv[None, :]).T.astype(ml_dtypes.bfloat16)
    WvT = (Wv * g_kv[None, :]).T.astype(ml_dtypes.bfloat16)
    bq_eff = (bq + Wq @ b_q).astype(np.float32)
    bv_eff = (bv + Wv @ b_kv).astype(np.float32)
    # The K bias (bk + Wk@b_kv) shifts all key scores of a query equally and
    # cancels in softmax; it is dropped.  The V bias passes through softmax
    # unchanged (weights sum to 1): fold into the final output bias.
    WoT = out_w.T.astype(ml_dtypes.bfloat16)
    bias_total = (out_b + out_w @ bv_eff).astype(np.float32)

    # per-query key mask; all-zero mask rows attend everywhere
    kv16 = kv.astype(ml_dtypes.bfloat16)
    allowed = (mask != 0)
    has_any = allowed.any(axis=-1, keepdims=True)
    eff = np.where(has_any, allowed, True)  # [B, NQ, HW] bool

    common = {
        "q": np.ascontiguousarray(q.astype(ml_dtypes.bfloat16)),
        "wqT": np.ascontiguousarray(WqT.reshape(NDC, 128, D).transpose(1, 0, 2)),
        "wkT": np.ascontiguousarray(WkT.reshape(NDC, 128, D).transpose(1, 0, 2)),
        "wvT": np.ascontiguousarray(WvT.reshape(NDC, 128, D).transpose(1, 0, 2)),
        "woT": np.ascontiguousarray(WoT.reshape(NDC, 128, D).transpose(1, 0, 2)),
        "biasq": np.ascontiguousarray(bq_eff.reshape(NDC, 128).T),
    }
    in_maps = []
    for c in range(NCORE):
        sl = slice(c * KC, (c + 1) * KC)
        kv_c = kv16[:, sl, :].reshape(B, NKT, 128, D)
        # mask slice -> [128, B, NKT, NQ] bf16 (keysub-tile on partitions)
        m_c = eff[:, :, sl].transpose(0, 2, 1).reshape(B, NKT, 128, NQ)
        m_c = m_c.transpose(2, 0, 1, 3).astype(ml_dtypes.bfloat16)
        in_maps.append({
            **common,
            "kv": np.ascontiguousarray(kv_c),
            "maskT": np.ascontiguousarray(m_c),
        })
    return in_maps, bias_total


def kernel(q, kv, mask, in_proj_w, in_proj_b, out_w, out_b, g_q, b_q, g_kv, b_kv):
    in_maps, bias_total = _prep_in_maps(
        q, kv, mask, in_proj_w, in_proj_b, out_w, out_b, g_q, b_q, g_kv, b_kv
    )
    if "nc" not in _compiled:
        _compiled["nc"] = _build()
    nc = _compiled["nc"]

    res = run_bass_kernel_spmd(nc, in_maps, core_ids=list(range(NCORE)))

    out = np.zeros((B, NQ, D), np.float32)
    for c in range(NCORE):
        part = res.results[c]["out"]  # [128 p, NDC m, B, NQ]; dout = m*128+p
        out += part.transpose(2, 3, 1, 0).reshape(B, NQ, D).astype(np.float32)
    out += bias_total[None, None, :]
    return out


# revision 37
# speedup vs baseline: 1.0508x; 1.0474x over previous
"""Masked cross-attention (B=4, NQ=100, HW=4096, D=1024, H=16) on 8 TRN2 cores.

Sharding: kv rows (keys) are split 8 ways; each core runs LayerNorm + K/V
projection on its 512-key slice per batch, computes unnormalized partial
attention for all (b, h) against its keys, all-reduces the softmax
denominators on device, normalizes, and computes a partial out-projection.
The host sums the 8 partial outputs and adds the folded bias.

Schedule: the Tensor-engine instruction stream is software-pipelined so the
PE never waits on ACT/DVE work — batch b's exp/mask runs under batch b+1's
K projection.  The denominator all-reduce is split in two (batches 0-1 and
2-3): the first hides fully under compute, and the tail of the second is
covered by the first half's combine/out-projection/output-DMA.

LayerNorm gamma/beta are folded into the projection weights/biases on the
host; the V-projection bias is folded into the final output bias (exact
because softmax weights sum to one) and the K-projection bias is dropped
entirely (it shifts every key score of a query equally, which softmax
cancels).
"""
import sys

sys.path.insert(0, "/opt/trn_rl_repo")

import numpy as np
import ml_dtypes

import concourse.bacc as bacc
import concourse.bass as bass
import concourse.mybir as mybir
import concourse.tile as tile
from concourse.bass_utils import run_bass_kernel_spmd
B, NQ, HW, D, H = 4, 100, 4096, 1024, 16
HD = D // H          # 64
NCORE = 8
KC = HW // NCORE     # 512 keys per core per batch
NKT = KC // 128      # 4 key sub-tiles of 128
NDC = D // 128       # 8 chunks of the model dim
EPS = 1e-5
SCALE = 1.0 / np.sqrt(np.float32(HD))  # 1/8

F32 = mybir.dt.float32
BF16 = mybir.dt.bfloat16
AF = mybir.ActivationFunctionType
ALU = mybir.AluOpType

_compiled = {}


def _build():
    nc = bacc.Bacc("TRN2", target_bir_lowering=False, num_devices=NCORE)

    kv_d = nc.dram_tensor("kv", [B, NKT, 128, D], BF16, kind="ExternalInput")
    q_d = nc.dram_tensor("q", [B, NQ, D], BF16, kind="ExternalInput")
    mask_d = nc.dram_tensor("maskT", [128, B, NKT, NQ], BF16, kind="ExternalInput")
    wq_d = nc.dram_tensor("wqT", [128, NDC, D], BF16, kind="ExternalInput")
    wk_d = nc.dram_tensor("wkT", [128, NDC, D], BF16, kind="ExternalInput")
    wv_d = nc.dram_tensor("wvT", [128, NDC, D], BF16, kind="ExternalInput")
    wo_d = nc.dram_tensor("woT", [128, NDC, D], BF16, kind="ExternalInput")
    bq_d = nc.dram_tensor("biasq", [128, NDC], F32, kind="ExternalInput")
    out_d = nc.dram_tensor("out", [128, NDC, B, NQ], BF16, kind="ExternalOutput")

    with tile.TileContext(nc) as tc:
        with (
            tc.tile_pool(name="sb", bufs=1) as sb,
            tc.tile_pool(name="ps", bufs=1, space="PSUM") as ps,
            tc.tile_pool(name="dram", bufs=1, space="DRAM") as dram,
        ):
            # ---- constants ----
            eps_t = sb.tile([128, 1], F32, tag="eps")
            nc.vector.memset(eps_t[:], EPS)

            # Weights laid out [128, NDC, D] on the host: one contiguous DMA
            # each.  DMA queue order is managed explicitly below.
            wk_sb = sb.tile([128, NDC, D], BF16, tag="wk")
            wv_sb = sb.tile([128, NDC, D], BF16, tag="wv")
            bqv_sb = sb.tile([128, NDC], F32, tag="bqv")
            bq_sb = [bqv_sb[:, j:j + 1] for j in range(NDC)]
            wq_sb = sb.tile([128, NDC, D], BF16, tag="wq", bufs=1, name="wq")
            wo_sb = sb.tile([128, NDC, D], BF16, tag="wo")

            def layernorm_to_bf16(x_bf16, xn_bf16, p):
                """(x - mean) * rsqrt(var + eps), row-wise over the free dim."""
                stats = sb.tile([128, 2, 6], F32, tag="lnstats", bufs=4)
                nc.vector.bn_stats(stats[:p, 0, :], x_bf16[:p, 0:512])
                nc.vector.bn_stats(stats[:p, 1, :], x_bf16[:p, 512:1024])
                mv = sb.tile([128, 2], F32, tag="lnmv", bufs=4)
                nc.vector.bn_aggr(mv[:p], stats[:p])
                rstd = sb.tile([128, 1], F32, tag="lnrstd", bufs=4)
                nc.scalar.activation(rstd[:p], mv[:p, 1:2], AF.Sqrt, bias=eps_t[:p])
                nc.vector.reciprocal(rstd[:p], rstd[:p])
                nc.vector.tensor_scalar(
                    xn_bf16[:p], x_bf16[:p], mv[:p, 0:1], rstd[:p],
                    ALU.subtract, ALU.mult,
                )

            sloc = [dram.tile([NQ, H], F32, tag=f"sloc{b}", name=f"sloc{b}")
                    for b in range(B)]
            sglob = [dram.tile([NQ, H], F32, tag=f"sglob{b}", name=f"sglob{b}")
                     for b in range(B)]
            # normalized, transposed context for all batches: [p, k, b, q]
            ctxT_all = sb.tile([128, B, NDC, 112], BF16, tag="ctxTall")
            NQP = 112  # q rows padded to the XBAR 16-row granule
            qnT = sb.tile([128, B, NDC, NQP], BF16, tag="qnT")
            qpT = []

            def load_kv(b):
                """kv loads only (grouped so later transpose waits on the
                sync queue never delay prefetch)."""
                kvraws = []
                for r in range(NKT):
                    kvraw = sb.tile([128, D], BF16, tag="kvraw", bufs=4)
                    nc.sync.dma_start(kvraw[:], kv_d[b, r])
                    kvraws.append(kvraw)
                return kvraws

            def transpose_kv(b, kvraws):
                """LayerNorm + XBAR DMA transpose into kvnT[b].

                kvnT layout: [128 dpart, NKT, NDC, 128 keys], one contiguous
                [NDC, 128] destination slab per key sub-tile so the hardware
                DMA transpose (sync queue) replaces PE transposes + DVE
                copies."""
                kvnT = sb.tile([128, NKT, NDC, 128], BF16, tag="kvnT", bufs=2,
                               name=f"kvnT_{b}")
                for r in range(NKT):
                    layernorm_to_bf16(kvraws[r], kvraws[r], 128)
                    nc.scalar.dma_start_transpose(kvnT[:, r], kvraws[r][:])
                return kvnT

            def kproj(b, kvnT):
                """K projection -> kpT[j]: [128 dout, KC keys] (no bias:
                the K bias cancels in softmax)."""
                kpT = []
                for j in range(NDC):
                    kpT.append(
                        sb.tile([128, KC], BF16, tag=f"kpT{j}", bufs=2,
                                name=f"kpT{j}_{b}")
                    )
                    acc = ps.tile([128, KC], F32, tag="mm", bufs=3)
                    for k in range(NDC):
                        nc.tensor.matmul(
                            acc[:],
                            lhsT=wk_sb[:, k, j * 128:(j + 1) * 128],
                            rhs=kvnT[:, :, k, :],
                            start=(k == 0), stop=(k == NDC - 1),
                        )
                    nc.scalar.activation(kpT[j][:], acc[:], AF.Copy)
                return kpT

            def vproj(b, kvnT):
                """V projection -> vp_ext[r]: [128 keys, H, HD+1], col HD=1.
                For the last batch all PSUM copies ride the DVE so the ACT
                queue is free for the exp chain that gates the final
                all-reduce."""
                vp_ext = []
                for r in range(NKT):
                    vpe = sb.tile([128, H, HD + 1], BF16, tag=f"vpe{r}", bufs=2,
                                  name=f"vpe{r}_{b}")
                    vp_ext.append(vpe)
                    nc.vector.memset(vpe[:, :, HD:HD + 1], 1.0)
                    for nh in range(2):
                        acc = ps.tile([128, 512], F32, tag="mm", bufs=3)
                        for k in range(NDC):
                            nc.tensor.matmul(
                                acc[:],
                                lhsT=kvnT[:, r, k, :],
                                rhs=wv_sb[:, k, nh * 512:(nh + 1) * 512],
                                start=(k == 0), stop=(k == NDC - 1),
                            )
                        if nh == 0 or b == B - 1:
                            nc.vector.tensor_copy(
                                out=vpe[:, nh * 8:(nh + 1) * 8, 0:HD],
                                in_=acc[:].rearrange("p (g d) -> p g d", g=8),
                            )
                        else:
                            nc.scalar.activation(
                                vpe[:, nh * 8:(nh + 1) * 8, 0:HD],
                                acc[:].rearrange("p (g d) -> p g d", g=8),
                                AF.Copy,
                            )
                return vp_ext

            def load_q():
                qraws = []
                for b in range(B):
                    qraw = sb.tile([NQ, D], BF16, tag="qraw", bufs=3)
                    nc.sync.dma_start(qraw[:], q_d[b])
                    qraws.append(qraw)
                return qraws

            def ln_T_q(qraws):
                """LayerNorm + XBAR transpose of q, all batches (rows
                100:112 of the padded tile are never read downstream)."""
                for b in range(B):
                    qn = sb.tile([NQP, D], BF16, tag="qn", bufs=1)
                    layernorm_to_bf16(qraws[b], qn, NQ)
                    nc.scalar.dma_start_transpose(qnT[:, b], qn[:])

            def qproj():
                """qpT_pad[j]: [128, B, 2, NQ] block-diagonal by head: rows
                0:64 hold head 2j over i=0 columns, rows 64:128 hold head
                2j+1 over i=1 columns, zeros elsewhere."""
                for j in range(NDC):
                    qpT.append(
                        sb.tile([128, B, 2, NQ], BF16, tag=f"qpT{j}",
                                name=f"qpT{j}")
                    )
                    nc.gpsimd.memset(qpT[j][:], 0.0)
                    acc = ps.tile([128, B * NQ], F32, tag="sc", bufs=2)
                    for k in range(NDC):
                        nc.tensor.matmul(
                            acc[:],
                            lhsT=wq_sb[:, k, j * 128:(j + 1) * 128],
                            rhs=qnT[:, :, k, 0:NQ],
                            start=(k == 0), stop=(k == NDC - 1),
                        )
                    nc.scalar.activation(
                        qpT[j][0:HD, :, 0, :],
                        acc[0:HD, :].rearrange("p (b q) -> p b q", b=B),
                        AF.Identity, bias=bq_sb[j][0:HD],
                    )
                    nc.scalar.activation(
                        qpT[j][HD:128, :, 1, :],
                        acc[HD:128, :].rearrange("p (b q) -> p b q", b=B),
                        AF.Identity, bias=bq_sb[j][HD:128],
                    )

            def scores_exp(b, kpT, mask_b):
                """scores^T + exp + mask for all heads of batch b."""
                exp_all = sb.tile([128, NKT, H, NQ], BF16, tag="expall", bufs=2,
                                  name=f"exp_{b}")
                for j in range(NDC):
                    for c2 in range(2):
                        sc = ps.tile([128, 2, 2, NQ], F32, tag="sc", bufs=2)
                        for cc in range(2):
                            c = 2 * c2 + cc
                            nc.tensor.matmul(
                                sc[:, cc, :, :].rearrange("p i q -> p (i q)"),
                                lhsT=kpT[j][:, c * 128:(c + 1) * 128],
                                rhs=qpT[j][:, b, :, :].rearrange(
                                    "p i q -> p (i q)"),
                                start=True, stop=True,
                            )
                        nc.scalar.activation(
                            exp_all[:, 2 * c2:2 * c2 + 2, 2 * j:2 * j + 2, :],
                            sc[:], AF.Exp, scale=float(SCALE),
                        )
                    for hh in range(2):
                        h = 2 * j + hh
                        eng = nc.vector if hh == 0 else nc.gpsimd
                        eng.tensor_mul(
                            exp_all[:, :, h, :], exp_all[:, :, h, :], mask_b
                        )
                return exp_all

            def ctx_block(b, exp_all, vp_ext):
                """Unnormalized ctx + denominators for batch b; DMA the local
                denominators out for the all-reduce."""
                ctx_b = sb.tile([NQ, H, HD + 1], F32, tag="ctxsb", bufs=4,
                                name=f"ctx_{b}")
                for h in range(H):
                    ctx_ps = ps.tile([NQ, HD + 1], F32, tag="small", bufs=3)
                    for c in range(NKT):
                        nc.tensor.matmul(
                            ctx_ps[:],
                            lhsT=exp_all[:, c, h, :],
                            rhs=vp_ext[c][:, h, :],
                            start=(c == 0), stop=(c == NKT - 1),
                        )
                    if h % 2 == 0 or b == B - 1:
                        nc.vector.tensor_copy(out=ctx_b[:, h, :], in_=ctx_ps[:])
                    else:
                        nc.scalar.activation(ctx_b[:, h, :], ctx_ps[:], AF.Copy)
                # pack the strided denominator column so its DMA moves 64B
                # runs instead of 4B elements (the DMA engine is descriptor-
                # latency-bound on tiny runs and would delay the all-reduce)
                den = sb.tile([NQ, H], F32, tag="den", bufs=2)
                nc.gpsimd.tensor_copy(out=den[:], in_=ctx_b[:, :, HD])
                nc.sync.dma_start(sloc[b][:], den[:])
                return ctx_b

            def allreduce_b(b):
                """Per-batch denominator all-reduce.  One collective per
                batch: the first fires ~100us before its result is needed and
                absorbs cross-core launch skew; the rest are cheap syncs.
                The recip DMA rides the sync queue so its completion wait
                never blocks the gpsimd queue."""
                nc.gpsimd.collective_compute(
                    "AllReduce", ALU.add,
                    replica_groups=[list(range(NCORE))],
                    ins=[sloc[b][:].opt()], outs=[sglob[b][:].opt()],
                )
                return b

            def recip_fetch(b):
                """Fetch the all-reduced denominators for batch b.  Must be
                emitted BEFORE any later collective_compute: DMA-after-
                collective ordering uses one program-order CC counter."""
                recip = sb.tile([NQ, H], F32, tag="recip", bufs=4,
                                name=f"recip_{b}")
                nc.scalar.dma_start(recip[:], sglob[b][:])
                return recip

            def combine_block(b, ctx_b, recip):
                """Normalize by global denominators and XBAR-transpose into
                ctxT_all.  Scales alternate DVE/ACT by head."""
                nc.vector.reciprocal(recip[:], recip[:])
                ctxn = sb.tile([112, H, HD], BF16, tag="ctxn", bufs=1)
                for h in range(H):
                    if h % 2 == 0:
                        nc.vector.tensor_scalar_mul(
                            ctxn[:NQ, h, :], ctx_b[:, h, 0:HD],
                            recip[:, h:h + 1]
                        )
                    else:
                        nc.scalar.activation(
                            ctxn[:NQ, h, :], ctx_b[:, h, 0:HD], AF.Copy,
                            scale=recip[:, h:h + 1],
                        )
                nc.scalar.dma_start_transpose(ctxT_all[:, b], ctxn[:])

            # reuses wq's slot (wq is dead after the Q projection)
            out_sb = sb.tile([128, NDC, B, NQ], BF16, tag="wq", bufs=1,
                             name="out_sb")

            def outproj(b0, nb):
                """Out-projection for batches [b0, b0+nb), then DMA that
                slice of the output."""
                for m in range(NDC):
                    acc = ps.tile([128, nb, NQ], F32, tag="sc", bufs=2)
                    for k in range(NDC):
                        nc.tensor.matmul(
                            acc[:],
                            lhsT=wo_sb[:, k, m * 128:(m + 1) * 128],
                            rhs=ctxT_all[:, b0:b0 + nb, k, 0:NQ],
                            start=(k == 0), stop=(k == NDC - 1),
                        )
                    if m % 2 == 0:
                        nc.vector.tensor_copy(
                            out=out_sb[:, m, b0:b0 + nb, :], in_=acc[:]
                        )
                    else:
                        nc.scalar.activation(
                            out_sb[:, m, b0:b0 + nb, :], acc[:], AF.Copy,
                        )
                nc.sync.dma_start(
                    out_d[:, :, b0:b0 + nb, :], out_sb[:, :, b0:b0 + nb, :]
                )

            mask_all = sb.tile([128, B, NKT, NQ], BF16, tag="maskb")

            def load_mask():
                nc.sync.dma_start(mask_all[:], mask_d[:])

            # ---- pipelined schedule ----
            # Sync queue: kv(b0), wk, q, wq, bq, wv, mask0 all dispatched
            # before the first (LN-gated) XBAR transpose so nothing blocks
            # prefetch.  PE queue: Kproj(b0), qT, Qproj, Vproj(b0), then per
            # batch: scores(b) | Kproj(b+1) | ctx(b) | Vproj(b+1).
            kvraws0 = load_kv(0)
            nc.sync.dma_start(wk_sb[:], wk_d[:])
            nc.sync.dma_start(wv_sb[:], wv_d[:])
            nc.sync.dma_start(bqv_sb[:], bq_d[:])
            nc.sync.dma_start(wq_sb[:], wq_d[:])
            load_mask()
            kvnT0 = transpose_kv(0, kvraws0)
            qraws = load_q()

            kpT0 = kproj(0, kvnT0)
            ln_T_q(qraws)
            qproj()
            kvraws1 = load_kv(1)
            nc.sync.dma_start(wo_sb[:], wo_d[:])
            kvnT1 = transpose_kv(1, kvraws1)
            vp0 = vproj(0, kvnT0)

            exp0 = scores_exp(0, kpT0, mask_all[:, 0])
            kpT1 = kproj(1, kvnT1)
            ctx0 = ctx_block(0, exp0, vp0)
            recip0 = allreduce_b(0)
            kvraws2 = load_kv(2)
            kvnT2 = transpose_kv(2, kvraws2)
            vp1 = vproj(1, kvnT1)

            exp1 = scores_exp(1, kpT1, mask_all[:, 1])
            kpT2 = kproj(2, kvnT2)
            ctx1 = ctx_block(1, exp1, vp1)
            recip1 = allreduce_b(1)
            kvraws3 = load_kv(3)
            kvnT3 = transpose_kv(3, kvraws3)
            vp2 = vproj(2, kvnT2)

            exp2 = scores_exp(2, kpT2, mask_all[:, 2])
            kpT3 = kproj(3, kvnT3)
            ctx2 = ctx_block(2, exp2, vp2)
            recip2 = allreduce_b(2)
            vp3 = vproj(3, kvnT3)

            exp3 = scores_exp(3, kpT3, mask_all[:, 3])
            r0 = recip_fetch(0)
            r1 = recip_fetch(1)
            r2 = recip_fetch(2)
            ctx3 = ctx_block(3, exp3, vp3)
            allreduce_b(3)
            combine_block(0, ctx0, r0)
            combine_block(1, ctx1, r1)
            outproj(0, 2)
            combine_block(2, ctx2, r2)
            outproj(2, 1)
            r3 = recip_fetch(3)
            combine_block(3, ctx3, r3)
            outproj(3, 1)

    nc.compile()
    return nc


def _prep_in_maps(q, kv, mask, in_proj_w, in_proj_b, out_w, out_b,
                  g_q, b_q, g_kv, b_kv):
    """Host-side prep: fold LN affine + V-bias, shard kv/mask per core.

    Returns (in_maps, bias_total)."""
    q = np.asarray(q, np.float32)
    kv = np.asarray(kv, np.float32)
    mask = np.asarray(mask)
    in_proj_w = np.asarray(in_proj_w, np.float32)
    in_proj_b = np.asarray(in_proj_b, np.float32)
    out_w = np.asarray(out_w, np.float32)
    out_b = np.asarray(out_b, np.float32)
    g_q = np.asarray(g_q, np.float32)
    b_q = np.asarray(b_q, np.float32)
    g_kv = np.asarray(g_kv, np.float32)
    b_kv = np.asarray(b_kv, np.float32)

    Wq, Wk, Wv = in_proj_w[:D], in_proj_w[D:2 * D], in_proj_w[2 * D:]
    bq, bk, bv = in_proj_b[:D], in_proj_b[D:2 * D], in_proj_b[2 * D:]

    # Fold LayerNorm affine into projections: LN(x)*g+b @ W^T + c
    #   = LN(x) @ (W*g)^T + (W@b + c)
    WqT = (Wq * g_q[None, :]).T.astype(ml_dtypes.bfloat16)
    WkT = (Wk * g_kv[None, :]).T.astype(ml_dtypes.bfloat16)
    WvT = (Wv * g_kv[None, :]).T.astype(ml_dtypes.bfloat16)
    bq_eff = (bq + Wq @ b_q).astype(np.float32)
    bv_eff = (bv + Wv @ b_kv).astype(np.float32)
    # The K bias (bk + Wk@b_kv) shifts all key scores of a query equally and
    # cancels in softmax; it is dropped.  The V bias passes through softmax
    # unchanged (weights sum to 1): fold into the final output bias.
    WoT = out_w.T.astype(ml_dtypes.bfloat16)
    bias_total = (out_b + out_w @ bv_eff).astype(np.float32)

    # per-query key mask; all-zero mask rows attend everywhere
    kv16 = kv.astype(ml_dtypes.bfloat16)
    allowed = (mask != 0)
    has_any = allowed.any(axis=-1, keepdims=True)
    eff = np.where(has_any, allowed, True)  # [B, NQ, HW] bool

    common = {
        "q": np.ascontiguousarray(q.astype(ml_dtypes.bfloat16)),
        "wqT": np.ascontiguousarray(WqT.reshape(NDC, 128, D).transpose(1, 0, 2)),
        "wkT": np.ascontiguousarray(WkT.reshape(NDC, 128, D).transpose(1, 0, 2)),
        "wvT": np.ascontiguousarray(WvT.reshape(NDC, 128, D).transpose(1, 0, 2)),
        "woT": np.ascontiguousarray(WoT.reshape(NDC, 128, D).transpose(1, 0, 2)),
        "biasq": np.ascontiguousarray(bq_eff.reshape(NDC, 128).T),
    }
    in_maps = []
    for c in range(NCORE):
        sl = slice(c * KC, (c + 1) * KC)
        kv_c = kv16[:, sl, :].reshape(B, NKT, 128, D)
        # mask slice -> [128, B, NKT, NQ] bf16 (keysub-tile on partitions)
        m_c = eff[:, :, sl].transpose(0, 2, 1).reshape(B, NKT, 128, NQ)
        m_c = m_c.transpose(2, 0, 1, 3).astype(ml_dtypes.bfloat16)
        in_maps.append({
            **common,
            "kv": np.ascontiguousarray(kv_c),
            "maskT": np.ascontiguousarray(m_c),
        })
    return in_maps, bias_total


def kernel(q, kv, mask, in_proj_w, in_proj_b, out_w, out_b, g_q, b_q, g_kv, b_kv):
    in_maps, bias_total = _prep_in_maps(
        q, kv, mask, in_proj_w, in_proj_b, out_w, out_b, g_q, b_q, g_kv, b_kv
    )
    if "nc" not in _compiled:
        _compiled["nc"] = _build()
    nc = _compiled["nc"]

    res = run_bass_kernel_spmd(nc, in_maps, core_ids=list(range(NCORE)))

    out = np.zeros((B, NQ, D), np.float32)
    for c in range(NCORE):
        part = res.results[c]["out"]  # [128 p, NDC m, B, NQ]; dout = m*128+p
        out += part.transpose(2, 3, 1, 0).reshape(B, NQ, D).astype(np.float32)
    out += bias_total[None, None, :]
    return out
